# revision 13
# baseline (speedup 1.0000x reference)
"""Trainium2 Bass kernel for a 16-head causal attention block.

Problem: B=4, S=2048, D_MODEL=2048, N_HEADS=16, D_HEAD=128, fp32 I/O.

Sharding (8 cores): core c handles batch b = c//2 and head-group g = c%2
(8 heads each).  Each core computes its heads' attention and the partial
output projection (sum over its 8 heads) for its batch; the host sums the
two head-group partials per batch and adds the output bias.

Per-core dataflow (fp32 PSUM accumulation everywhere):
  emission order: h0-QK-proj, h1-QK-proj, V-proj(all), h0..h7 scores
  (h2+ include their QK proj), output projection.  This front-loads two
  heads' QK work so the PE starts on a 4MB xt8 DMA instead of the full
  16MB stream, and stretches the ACT(exp) window.

  QK proj (fp8 DoubleRow): qT/kT [dh, seq] bf16 via ACT identity+bias.
  scores, per q-block j (512 wide), k-tile PAIRS (2x128):
    ST pair [128,2,512] fp32 PSUM   (2 matmuls, bf16 operands)
    PT pair = exp(ST)               (ONE pair-wide ACT op)
      j==0 -> bf16 PT; j>=1 -> fp8e4 PT
    causal mask per diagonal sub-tile (DVE mul)
    acc += PT sub-tiles             (DVE, bf16 accumulator)
    j==0: attnT += v_tile^T (x) PT        (per-tile bf16 matmul)
    j>=1: attnT += v8_pair^T (x) PT_pair  (fp8 DoubleRow matmul)
  denom = partition_all_reduce(acc)  (GPSIMD, bf16 in / fp32 out)
  recip = approx 1/denom             (DVE)
  attall = aps * recip (bf16; j>=1 folds the 1/32 v-scale via
           scalar_tensor_tensor)
  V proj keeps x in bf16 (fp8 V projection fails the 2e-2 gate); the
  STORED v is fp8(32*v) for j>=1 plus bf16 tiles 0..3 for j==0 --
  hybrid validated at rel_err 3.4e-3 == baseline.
  out[p, m] = sum_h attall_h^T (x) Ow_h  (bf16 matmuls) -> fp32 -> DRAM
"""

import math
import sys

import numpy as np
import ml_dtypes

for _p in ("/opt/trn_rl_repo", "/root/.axon_site/_ro/trn_rl_repo"):
    if _p not in sys.path:
        sys.path.insert(0, _p)

BF16 = ml_dtypes.bfloat16
FP8 = ml_dtypes.float8_e4m3

# fp8(e4m3) DoubleRow for the Q/K projections; logits are tiny so fp8
# noise there is harmless.  SX/SW are undone inside the exp scale.
SX = 8.0
SW = 2000.0
SV = 32.0  # stored-v scale for the fp8 PV path

S_FULL = 2048
D_FULL = 2048
NH_LOC = 8  # heads per core
DH = 128
QB = 512  # q block width
N_CORES = 8

# DoubleRowSwInterleave for QK proj: weights pre-interleaved host-side so
# LDWEIGHTS reads contiguously (FWL-speed) instead of the +72% DR pattern.
USE_SWI = False


def build_program(seq=S_FULL, d_model=D_FULL, n_heads=NH_LOC, loop_n=1):
    import concourse.tile as tile
    from concourse import bacc, bass_isa, mybir

    f32 = mybir.dt.float32
    bf16 = mybir.dt.bfloat16
    fp8 = mybir.dt.float8e4
    AF = mybir.ActivationFunctionType
    DR = (
        mybir.MatmulPerfMode.DoubleRowSwInterleave
        if USE_SWI
        else mybir.MatmulPerfMode.DoubleRow
    )
    DRPV = mybir.MatmulPerfMode.DoubleRow
    MULT = mybir.AluOpType.mult
    ADD = mybir.AluOpType.add

    nt = d_model // 128  # contraction (d_model) tiles
    npt = seq // 128  # seq tiles (p / k)
    nqb = seq // QB  # q blocks
    kt_per_qb = QB // 128  # 4
    nhd = n_heads * DH  # concatenated head width
    nblk = nhd // 512  # 512-wide chunks of (h, d)

    nc = bacc.Bacc(
        "TRN2", target_bir_lowering=False, debug=False, enable_asserts=False
    )

    exp_scale = 1.0 / ((SX * SW) ** 2 * math.sqrt(DH))
    xt_d = nc.dram_tensor("xt", [128, nt, seq], bf16, kind="ExternalInput").ap()
    xt8_d = nc.dram_tensor("xt8", [128, nt, seq], fp8, kind="ExternalInput").ap()
    if USE_SWI:
        qw_d = nc.dram_tensor(
            "qw", [n_heads, 128, nt // 2, 256], fp8, kind="ExternalInput"
        ).ap()
        kw_d = nc.dram_tensor(
            "kw", [n_heads, 128, nt // 2, 256], fp8, kind="ExternalInput"
        ).ap()
    else:
        qw_d = nc.dram_tensor(
            "qw", [n_heads, 128, nt, 128], fp8, kind="ExternalInput"
        ).ap()
        kw_d = nc.dram_tensor(
            "kw", [n_heads, 128, nt, 128], fp8, kind="ExternalInput"
        ).ap()
    vw_d = nc.dram_tensor("vw", [128, nt, nhd], bf16, kind="ExternalInput").ap()
    ow_d = nc.dram_tensor("ow", [n_heads, 128, d_model], bf16, kind="ExternalInput").ap()
    qb_d = nc.dram_tensor("qb", [128, n_heads], f32, kind="ExternalInput").ap()
    kb_d = nc.dram_tensor("kb", [128, n_heads], f32, kind="ExternalInput").ap()
    vb_d = nc.dram_tensor("vb", [128, nhd], f32, kind="ExternalInput").ap()
    vb32_d = nc.dram_tensor("vb32", [128, nhd], f32, kind="ExternalInput").ap()
    mask_d = nc.dram_tensor("mask", [128, 896], bf16, kind="ExternalInput").ap()
    mask8_d = nc.dram_tensor("mask8", [128, 896], fp8, kind="ExternalInput").ap()
    out_d = nc.dram_tensor("out", [seq, d_model], f32, kind="ExternalOutput").ap()

    from concourse import library_config

    with tile.TileContext(nc) as tc:
        nc.gpsimd.load_library(library_config.attn)
        for _rep in range(loop_n):
            # PSUM pools: 2 + 4 + 2 = 8 banks
            mm_ps = tc.alloc_tile_pool(name="mmps", bufs=2, space="PSUM")
            st_ps = tc.alloc_tile_pool(name="stps", bufs=2, space="PSUM")
            pv_ps = tc.alloc_tile_pool(name="pvps", bufs=2, space="PSUM")

            consts = tc.alloc_tile_pool(name="consts", bufs=1)
            xt8_pool = tc.alloc_tile_pool(name="xt8p", bufs=1)
            v8_pool = tc.alloc_tile_pool(name="v8p", bufs=1)
            vb16_pool = tc.alloc_tile_pool(name="vb16p", bufs=1)
            w_pool = tc.alloc_tile_pool(name="wp", bufs=3)
            qk_pool = tc.alloc_tile_pool(name="qkp", bufs=3)
            xt_pool = tc.alloc_tile_pool(name="xtp", bufs=1)
            vw_pool = tc.alloc_tile_pool(name="vwp", bufs=1)

            mask_sb = consts.tile([128, 896], bf16)
            mask8_sb = consts.tile([128, 896], fp8)
            qb_sb = consts.tile([128, n_heads], f32)
            kb_sb = consts.tile([128, n_heads], f32)
            vb_sb = consts.tile([128, nhd], f32)
            vb32_sb = consts.tile([128, nhd], f32)

            xt8 = xt8_pool.tile([128, nt, seq], fp8)
            vall8 = v8_pool.tile([128, npt, nhd], fp8)
            vallb = vb16_pool.tile([128, kt_per_qb, nhd], bf16)
            xt = xt_pool.tile([128, nt, seq], bf16)
            vw = vw_pool.tile([128, nt, nhd], bf16)

            # ---------------- DMA: xt8 + early-head weights first ----------
            for t in range(nt):
                nc.sync.dma_start(xt8[:, t, :], xt8_d[:, t, :])
            nc.sync.dma_start(qb_sb, qb_d)
            nc.sync.dma_start(kb_sb, kb_d)
            head_w = {}

            def fetch_w(h):
                wq = w_pool.tile(list(qw_d[h].shape), fp8, tag="wq", name=f"wq_{h}")
                nc.sync.dma_start(wq, qw_d[h])
                wk = w_pool.tile(list(kw_d[h].shape), fp8, tag="wk", name=f"wk_{h}")
                nc.sync.dma_start(wk, kw_d[h])
                head_w[h] = (wq, wk)

            for h in (0, 1):
                fetch_w(h)
            for t in range(nt):
                nc.sync.dma_start(xt[:, t, :], xt_d[:, t, :])
                nc.sync.dma_start(vw[:, t, :], vw_d[:, t, :])
            nc.sync.dma_start(vb_sb, vb_d)
            nc.sync.dma_start(vb32_sb, vb32_d)
            nc.sync.dma_start(mask_sb, mask_d)
            nc.sync.dma_start(mask8_sb, mask8_d)

            # ---------------- QK projection (fp8 DoubleRow) -----------------
            qk_cache = {}

            def qk_proj_chunks(h):
                """Generator emitting 16 single-PSUM chunks (8 DR matmuls +
                one ACT identity each) of head h's Q/K projection.  Chunks
                alternate the two mm PSUM buffers, so chunk c+1 never waits
                on chunk c's ACT drain."""
                wq, wk = head_w[h]
                qT = qk_pool.tile([128, nqb, 512], bf16, tag="qT", name=f"qT_{h}")
                kT = qk_pool.tile([128, nqb, 512], bf16, tag="kT", name=f"kT_{h}")
                qk_cache[h] = (qT, kT)
                for w_t, dst, b_sb in ((wq, qT, qb_sb), (wk, kT, kb_sb)):
                    for pb in range(nqb):
                        ps = mm_ps.tile(
                            [128, 512], f32, tag="mm",
                            name=f"{dst.name[:2]}_{h}_{pb}",
                        )
                        for m2 in range(nt // 2):
                            lhs = w_t[:, m2, :] if USE_SWI else w_t[:, 2 * m2 : 2 * m2 + 2, :]
                            nc.tensor.matmul(
                                ps,
                                lhs,
                                xt8[:, 2 * m2 : 2 * m2 + 2,
                                    pb * 512 : (pb + 1) * 512],
                                start=(m2 == 0),
                                stop=(m2 == nt // 2 - 1),
                                perf_mode=DR,
                            )
                        nc.scalar.activation(
                            dst[:, pb, :],
                            ps,
                            AF.Identity,
                            bias=b_sb[:, h : h + 1],
                        )
                        yield
                head_w.pop(h, None)

            def run_all(gen):
                if gen is not None:
                    for _ in gen:
                        pass

            run_all(qk_proj_chunks(0))

            # ---------------- V projection (bf16), fp8 + bf16 stores --------
            p1_pools = [mm_ps, pv_ps]
            for p_i in range(npt):
                for blk in range(nblk):
                    pidx = (p_i * nblk + blk) % 2
                    vps = p1_pools[pidx].tile(
                        [128, 512], f32, tag=["mm", "pv"][pidx],
                        name=f"vps_{p_i}_{blk}",
                    )
                    for m in range(nt):
                        nc.tensor.matmul(
                            vps,
                            xt[:, m, p_i * 128 : (p_i + 1) * 128],
                            vw[:, m, blk * 512 : (blk + 1) * 512],
                            start=(m == 0),
                            stop=(m == nt - 1),
                        )
                    cols = slice(blk * 512, (blk + 1) * 512)
                    nc.vector.scalar_tensor_tensor(
                        vall8[:, p_i, cols], vps, SV, vb32_sb[:, cols], MULT, ADD
                    )
                    if p_i < kt_per_qb:
                        nc.vector.tensor_add(
                            vallb[:, p_i, cols], vps, vb_sb[:, cols]
                        )
            vw_pool.release()
            xt_pool.release()

            att_pool = tc.alloc_tile_pool(name="attp", bufs=1, side="right")
            pt_pool = tc.alloc_tile_pool(name="ptp", bufs=3)
            ptb_pool = tc.alloc_tile_pool(name="ptbp", bufs=2)
            acc_pool = tc.alloc_tile_pool(name="accp", bufs=3)
            rc_pool = tc.alloc_tile_pool(name="rcp", bufs=2)
            rb_pool = tc.alloc_tile_pool(name="rbp", bufs=2)
            attall = att_pool.tile([128, n_heads, seq], bf16)

            # ---------------- per-head attention ----------------------------
            # Head h's score blocks interleave head (h+1)'s QK-projection
            # chunks: when the score chain stalls on ACT/DVE, the in-order
            # PE queue still has projection matmuls to chew on.
            for h in range(n_heads):
                qT, kT = qk_cache.pop(h)
                nxt = h + 1
                if nxt < n_heads:
                    if nxt + 1 < n_heads:
                        fetch_w(nxt + 1)  # DMA for the head after next
                    ileave = qk_proj_chunks(nxt)
                else:
                    ileave = None

                for j in range(nqb):
                    nk = (j + 1) * kt_per_qb
                    aps = pv_ps.tile([128, 512], f32, tag="pv", name=f"aps_{h}_{j}")
                    acc = acc_pool.tile([128, 512], bf16, tag="acc", name=f"acc_{h}_{j}")
                    use8 = j >= 1
                    for ip in range(nk // 2):
                        i0, i1 = 2 * ip, 2 * ip + 1
                        s0 = 128 * (i0 - kt_per_qb * j) if i0 >= kt_per_qb * j else 0
                        s1 = 128 * (i1 - kt_per_qb * j) if i1 >= kt_per_qb * j else 0
                        stp = st_ps.tile(
                            [128, 2, 512], f32, tag="st", name=f"stp_{h}_{j}_{ip}"
                        )
                        for t, i in ((0, i0), (1, i1)):
                            nc.tensor.matmul(
                                stp[:, t, s0:512],
                                kT[:, i // 4, (i % 4) * 128 : (i % 4 + 1) * 128],
                                qT[:, j, s0:512],
                                start=True,
                                stop=True,
                            )
                        if use8:
                            ptile = pt_pool.tile(
                                [128, 2, 512], fp8, tag="pt", name=f"pt_{h}_{j}_{ip}"
                            )
                            msk = mask8_sb
                        else:
                            ptile = ptb_pool.tile(
                                [128, 2, 512], bf16, tag="ptb", name=f"pt_{h}_{j}_{ip}"
                            )
                            msk = mask_sb
                        nc.scalar.activation(
                            ptile[:, :, s0:512], stp[:, :, s0:512], AF.Exp,
                            scale=exp_scale,
                        )
                        for t, s in ((0, s0), (1, s1)):
                            if 2 * ip + t >= kt_per_qb * j:
                                nc.vector.tensor_mul(
                                    ptile[:, t, s0:512],
                                    ptile[:, t, s0:512],
                                    msk[:, 384 - (s - s0) : 896 - s],
                                )
                        if ip == 0:
                            nc.vector.tensor_add(
                                acc, ptile[:, 0, :], ptile[:, 1, :]
                            )
                        else:
                            nc.vector.tensor_add(
                                acc[:, s0:512], acc[:, s0:512], ptile[:, 0, s0:512]
                            )
                            nc.vector.tensor_add(
                                acc[:, s0:512], acc[:, s0:512], ptile[:, 1, s0:512]
                            )
                        if use8:
                            nc.tensor.matmul(
                                aps[:, s0:512],
                                vall8[:, i0 : i0 + 2, h * 128 : (h + 1) * 128],
                                ptile[:, :, s0:512],
                                start=(ip == 0),
                                stop=(ip == nk // 2 - 1),
                                perf_mode=DRPV,
                            )
                        else:
                            for t, (i, s) in ((0, (i0, s0)), (1, (i1, s1))):
                                nc.tensor.matmul(
                                    aps[:, s:512],
                                    vallb[:, i, h * 128 : (h + 1) * 128],
                                    ptile[:, t, s:512],
                                    start=(i == 0),
                                    stop=(i == nk - 1),
                                )
                    dnr = rb_pool.tile([128, 512], f32, tag="rb", name=f"dnr_{h}_{j}")
                    nc.gpsimd.partition_all_reduce(
                        dnr, acc, 128, bass_isa.ReduceOp.add
                    )
                    rc = rc_pool.tile([128, 512], f32, tag="rc", name=f"rc_{h}_{j}")
                    nc.vector.reciprocal_approx_fast(rc, dnr)
                    dst = attall[:, h, j * 512 : (j + 1) * 512]
                    if use8:
                        nc.vector.scalar_tensor_tensor(
                            dst, aps, 1.0 / SV, rc, MULT, MULT
                        )
                    else:
                        nc.vector.tensor_mul(dst, aps, rc)
                    if ileave is not None:
                        # 4 projection chunks of head h+1 after each j block
                        for _ in range(4):
                            next(ileave, None)
            rb_pool.release()
            rc_pool.release()
            acc_pool.release()
            ptb_pool.release()
            pt_pool.release()
            qk_pool.release()
            w_pool.release()
            vb16_pool.release()
            v8_pool.release()
            xt8_pool.release()

            # ---------------- output projection -----------------------------
            ow_pool = tc.alloc_tile_pool(name="owp", bufs=2 * n_heads, side="right")
            osb_pool = tc.alloc_tile_pool(name="osbp", bufs=4, side="right")
            for mb in range(d_model // 512):
                ows = []
                for h in range(n_heads):
                    owt = ow_pool.tile([128, 512], bf16, tag="ow", name=f"ow_{mb}_{h}")
                    nc.sync.dma_start(owt, ow_d[h][:, mb * 512 : (mb + 1) * 512])
                    ows.append(owt)
                for p_i in range(npt):
                    ops = mm_ps.tile([128, 512], f32, tag="mm", name=f"ops_{mb}_{p_i}")
                    for h in range(n_heads):
                        nc.tensor.matmul(
                            ops,
                            attall[:, h, p_i * 128 : (p_i + 1) * 128],
                            ows[h],
                            start=(h == 0),
                            stop=(h == n_heads - 1),
                        )
                    osb = osb_pool.tile([128, 512], f32, tag="osb", name=f"osb_{mb}_{p_i}")
                    nc.scalar.copy(osb, ops)
                    nc.sync.dma_start(
                        out_d[p_i * 128 : (p_i + 1) * 128, mb * 512 : (mb + 1) * 512],
                        osb,
                    )
            osb_pool.release()
            ow_pool.release()
            att_pool.release()
            consts.release()
            pv_ps.release()
            st_ps.release()
            mm_ps.release()
    nc.finalize()
    return nc


def _interleave_pairs(w):
    """[nt, 128, 128] -> [nt//2, 128, 256] SwInterleave layout:
    flat[:, 2j] = A[:, 127-j], flat[:, 2j+1] = B[:, 127-j]."""
    nt = w.shape[0]
    out = np.empty((nt // 2, 128, 256), w.dtype)
    for t2 in range(nt // 2):
        a, b = w[2 * t2], w[2 * t2 + 1]
        out[t2, :, 0::2] = a[:, ::-1]
        out[t2, :, 1::2] = b[:, ::-1]
    return out


def make_core_inputs(x_b, Qw, Qb, Kw, Kb, Vw, Vb, Ow, seq, d_model, n_heads):
    """Host-side prep of one core's input map.

    x_b: [seq, d_model] fp32.  Qw/Kw/Vw: [n_heads, d_model, 128].
    Qb/Kb/Vb: [n_heads, 128].  Ow: [n_heads, 128, d_model].
    """
    nt = d_model // 128
    nhd = n_heads * DH

    # xT as [128(m_in), nt, seq]
    xTr = x_b.T.reshape(nt, 128, seq).transpose(1, 0, 2)
    xt = np.ascontiguousarray(xTr.astype(BF16))
    xt8 = np.ascontiguousarray((xTr * SX).astype(FP8))
    qw_t = (Qw * SW).reshape(n_heads, nt, 128, 128)
    kw_t = (Kw * SW).reshape(n_heads, nt, 128, 128)
    if USE_SWI:
        qw = np.ascontiguousarray(
            np.stack([_interleave_pairs(qw_t[h].astype(FP8)) for h in range(n_heads)])
        ).transpose(0, 2, 1, 3)
        qw = np.ascontiguousarray(qw)
        kw = np.ascontiguousarray(
            np.stack([_interleave_pairs(kw_t[h].astype(FP8)) for h in range(n_heads)])
        ).transpose(0, 2, 1, 3)
        kw = np.ascontiguousarray(kw)
    else:
        qw = np.ascontiguousarray(qw_t.transpose(0, 2, 1, 3).astype(FP8))
        kw = np.ascontiguousarray(kw_t.transpose(0, 2, 1, 3).astype(FP8))
    # v weights as [128(m_in), nt, (h d)]
    vw = np.ascontiguousarray(
        Vw.transpose(1, 0, 2)
        .reshape(d_model, nhd)
        .reshape(nt, 128, nhd)
        .transpose(1, 0, 2)
        .astype(BF16)
    )
    ow = np.ascontiguousarray(Ow.astype(BF16))
    qb = np.ascontiguousarray((Qb * SX * SW).T.astype(np.float32))
    kb = np.ascontiguousarray((Kb * SX * SW).T.astype(np.float32))
    vb = np.ascontiguousarray(
        np.broadcast_to(Vb.reshape(1, nhd), (128, nhd)).astype(np.float32)
    )
    vb32 = np.ascontiguousarray(vb * SV)
    r = np.arange(128, dtype=np.int64)[:, None]
    u = np.arange(896, dtype=np.int64)[None, :]
    mask = (r <= u - 384).astype(BF16)
    mask8 = mask.astype(FP8)
    return {
        "xt": xt,
        "xt8": xt8,
        "qw": qw,
        "kw": kw,
        "vw": vw,
        "ow": ow,
        "qb": qb,
        "kb": kb,
        "vb": vb,
        "vb32": vb32,
        "mask": mask,
        "mask8": mask8,
    }


_NC_CACHE = None


def kernel(**inputs):
    global _NC_CACHE
    from concourse.bass_utils import run_bass_kernel_spmd

    x = np.asarray(inputs["x"], np.float32)
    Q_w = np.asarray(inputs["Q_w"], np.float32)
    Q_b = np.asarray(inputs["Q_b"], np.float32)
    K_w = np.asarray(inputs["K_w"], np.float32)
    K_b = np.asarray(inputs["K_b"], np.float32)
    V_w = np.asarray(inputs["V_w"], np.float32)
    V_b = np.asarray(inputs["V_b"], np.float32)
    O_w = np.asarray(inputs["O_w"], np.float32)
    O_b = np.asarray(inputs["O_b"], np.float32)

    B, seq, d_model = x.shape

    if _NC_CACHE is None:
        _NC_CACHE = build_program(seq=seq, d_model=d_model, n_heads=NH_LOC)
    nc = _NC_CACHE

    in_maps = []
    for c in range(N_CORES):
        b = c // 2
        g = c % 2
        hs = slice(g * NH_LOC, (g + 1) * NH_LOC)
        in_maps.append(
            make_core_inputs(
                x[b], Q_w[hs], Q_b[hs], K_w[hs], K_b[hs], V_w[hs], V_b[hs],
                O_w[hs], seq, d_model, NH_LOC,
            )
        )

    res = run_bass_kernel_spmd(nc, in_maps, core_ids=list(range(N_CORES)))
    out = np.empty((B, seq, d_model), np.float32)
    for b in range(B):
        out[b] = res.results[2 * b]["out"] + res.results[2 * b + 1]["out"] + O_b[None, :]
    return out


# revision 14
# speedup vs baseline: 1.0751x; 1.0751x over previous
"""Trainium2 Bass kernel for a 16-head causal attention block.

Problem: B=4, S=2048, D_MODEL=2048, N_HEADS=16, D_HEAD=128, fp32 I/O.

Sharding (8 cores): core c handles batch b = c//2 and head-group g = c%2
(8 heads each).  Each core computes its heads' attention and the partial
output projection (sum over its 8 heads) for its batch; the host sums the
two head-group partials per batch and adds the output bias.

Per-core dataflow (fp32 PSUM accumulation everywhere):
  emission order: h0-QK-proj, h1-QK-proj, V-proj(all), h0..h7 scores
  (h2+ include their QK proj), output projection.  This front-loads two
  heads' QK work so the PE starts on a 4MB xt8 DMA instead of the full
  16MB stream, and stretches the ACT(exp) window.

  QK proj (fp8 DoubleRow): qT/kT [dh, seq] bf16 via ACT identity+bias.
  scores, per q-block j (512 wide), k-tile PAIRS (2x128):
    ST pair [128,2,512] fp32 PSUM   (2 matmuls, bf16 operands)
    PT pair = exp(ST)               (ONE pair-wide ACT op)
      j==0 -> bf16 PT; j>=1 -> fp8e4 PT
    causal mask per diagonal sub-tile (DVE mul)
    acc += PT sub-tiles             (DVE, bf16 accumulator)
    j==0: attnT += v_tile^T (x) PT        (per-tile bf16 matmul)
    j>=1: attnT += v8_pair^T (x) PT_pair  (fp8 DoubleRow matmul)
  denom = partition_all_reduce(acc)  (GPSIMD, bf16 in / fp32 out)
  recip = approx 1/denom             (DVE)
  attall = aps * recip (bf16; j>=1 folds the 1/32 v-scale via
           scalar_tensor_tensor)
  V proj keeps x in bf16 (fp8 V projection fails the 2e-2 gate); the
  STORED v is fp8(32*v) for j>=1 plus bf16 tiles 0..3 for j==0 --
  hybrid validated at rel_err 3.4e-3 == baseline.
  out[p, m] = sum_h attall_h^T (x) Ow_h  (bf16 matmuls) -> fp32 -> DRAM
"""

import math
import sys

import numpy as np
import ml_dtypes

for _p in ("/opt/trn_rl_repo", "/root/.axon_site/_ro/trn_rl_repo"):
    if _p not in sys.path:
        sys.path.insert(0, _p)

BF16 = ml_dtypes.bfloat16
FP8 = ml_dtypes.float8_e4m3

# fp8(e4m3) DoubleRow for the Q/K projections; logits are tiny so fp8
# noise there is harmless.  SX/SW are undone inside the exp scale.
SX = 8.0
SW = 2000.0
SV = 32.0  # stored-v scale for the fp8 PV path

S_FULL = 2048
D_FULL = 2048
NH_LOC = 8  # heads per core
DH = 128
QB = 512  # q block width
N_CORES = 8

# DoubleRowSwInterleave for QK proj: weights pre-interleaved host-side so
# LDWEIGHTS reads contiguously (FWL-speed) instead of the +72% DR pattern.
USE_SWI = False


def build_program(seq=S_FULL, d_model=D_FULL, n_heads=NH_LOC, loop_n=1):
    import concourse.tile as tile
    from concourse import bacc, bass_isa, mybir

    # Calibrate the scheduler's PE cost model to measured HW: matmuls
    # stream at ~0.5 ns/col (P0 downclock) and fp8-DoubleRow runs at the
    # same per-column rate as bf16 (the model prices DR at 0.5 cyc/row).
    # PE_CYCLE=1/1.2GHz makes bf16 ~1.5x over-priced and DR ~1.3x
    # under-priced, which ranks PE/DVE/ACT correctly for the head loop --
    # the Tile list-scheduler then orders the streams for a PE-bound
    # machine instead of a DVE-bound one.
    from concourse import hw_specs

    hw_specs.TRN2Spec.PE_CYCLE = 1e9 / 1.2e9

    f32 = mybir.dt.float32
    bf16 = mybir.dt.bfloat16
    fp8 = mybir.dt.float8e4
    AF = mybir.ActivationFunctionType
    DR = (
        mybir.MatmulPerfMode.DoubleRowSwInterleave
        if USE_SWI
        else mybir.MatmulPerfMode.DoubleRow
    )
    DRPV = mybir.MatmulPerfMode.DoubleRow
    MULT = mybir.AluOpType.mult
    ADD = mybir.AluOpType.add

    nt = d_model // 128  # contraction (d_model) tiles
    npt = seq // 128  # seq tiles (p / k)
    nqb = seq // QB  # q blocks
    kt_per_qb = QB // 128  # 4
    nhd = n_heads * DH  # concatenated head width
    nblk = nhd // 512  # 512-wide chunks of (h, d)

    nc = bacc.Bacc(
        "TRN2", target_bir_lowering=False, debug=False, enable_asserts=False
    )

    exp_scale = 1.0 / ((SX * SW) ** 2 * math.sqrt(DH))
    xt_d = nc.dram_tensor("xt", [128, nt, seq], bf16, kind="ExternalInput").ap()
    xt8_d = nc.dram_tensor("xt8", [128, nt, seq], fp8, kind="ExternalInput").ap()
    if USE_SWI:
        qw_d = nc.dram_tensor(
            "qw", [n_heads, 128, nt // 2, 256], fp8, kind="ExternalInput"
        ).ap()
        kw_d = nc.dram_tensor(
            "kw", [n_heads, 128, nt // 2, 256], fp8, kind="ExternalInput"
        ).ap()
    else:
        qw_d = nc.dram_tensor(
            "qw", [n_heads, 128, nt, 128], fp8, kind="ExternalInput"
        ).ap()
        kw_d = nc.dram_tensor(
            "kw", [n_heads, 128, nt, 128], fp8, kind="ExternalInput"
        ).ap()
    vw_d = nc.dram_tensor("vw", [128, nt, nhd], bf16, kind="ExternalInput").ap()
    ow_d = nc.dram_tensor("ow", [n_heads, 128, d_model], bf16, kind="ExternalInput").ap()
    qb_d = nc.dram_tensor("qb", [128, n_heads], f32, kind="ExternalInput").ap()
    kb_d = nc.dram_tensor("kb", [128, n_heads], f32, kind="ExternalInput").ap()
    vb_d = nc.dram_tensor("vb", [128, nhd], f32, kind="ExternalInput").ap()
    vb32_d = nc.dram_tensor("vb32", [128, nhd], f32, kind="ExternalInput").ap()
    mask_d = nc.dram_tensor("mask", [128, 896], bf16, kind="ExternalInput").ap()
    mask8_d = nc.dram_tensor("mask8", [128, 896], fp8, kind="ExternalInput").ap()
    out_d = nc.dram_tensor("out", [seq, d_model], f32, kind="ExternalOutput").ap()

    from concourse import library_config

    with tile.TileContext(nc) as tc:
        nc.gpsimd.load_library(library_config.attn)
        for _rep in range(loop_n):
            # PSUM pools: 2 + 4 + 2 = 8 banks
            mm_ps = tc.alloc_tile_pool(name="mmps", bufs=2, space="PSUM")
            st_ps = tc.alloc_tile_pool(name="stps", bufs=2, space="PSUM")
            pv_ps = tc.alloc_tile_pool(name="pvps", bufs=2, space="PSUM")

            consts = tc.alloc_tile_pool(name="consts", bufs=1)
            xt8_pool = tc.alloc_tile_pool(name="xt8p", bufs=1)
            v8_pool = tc.alloc_tile_pool(name="v8p", bufs=1)
            vb16_pool = tc.alloc_tile_pool(name="vb16p", bufs=1)
            w_pool = tc.alloc_tile_pool(name="wp", bufs=3)
            qk_pool = tc.alloc_tile_pool(name="qkp", bufs=3)
            xt_pool = tc.alloc_tile_pool(name="xtp", bufs=1)
            vw_pool = tc.alloc_tile_pool(name="vwp", bufs=1)

            mask_sb = consts.tile([128, 896], bf16)
            mask8_sb = consts.tile([128, 896], fp8)
            qb_sb = consts.tile([128, n_heads], f32)
            kb_sb = consts.tile([128, n_heads], f32)
            vb_sb = consts.tile([128, nhd], f32)
            vb32_sb = consts.tile([128, nhd], f32)

            xt8 = xt8_pool.tile([128, nt, seq], fp8)
            vall8 = v8_pool.tile([128, npt, nhd], fp8)
            vallb = vb16_pool.tile([128, kt_per_qb, nhd], bf16)
            xt = xt_pool.tile([128, nt, seq], bf16)
            vw = vw_pool.tile([128, nt, nhd], bf16)

            # ---------------- DMA: xt8 + early-head weights first ----------
            for t in range(nt):
                nc.sync.dma_start(xt8[:, t, :], xt8_d[:, t, :])
            nc.sync.dma_start(qb_sb, qb_d)
            nc.sync.dma_start(kb_sb, kb_d)
            head_w = {}

            def fetch_w(h):
                wq = w_pool.tile(list(qw_d[h].shape), fp8, tag="wq", name=f"wq_{h}")
                nc.sync.dma_start(wq, qw_d[h])
                wk = w_pool.tile(list(kw_d[h].shape), fp8, tag="wk", name=f"wk_{h}")
                nc.sync.dma_start(wk, kw_d[h])
                head_w[h] = (wq, wk)

            for h in (0, 1):
                fetch_w(h)
            for t in range(nt):
                nc.sync.dma_start(xt[:, t, :], xt_d[:, t, :])
                nc.sync.dma_start(vw[:, t, :], vw_d[:, t, :])
            nc.sync.dma_start(vb_sb, vb_d)
            nc.sync.dma_start(vb32_sb, vb32_d)
            nc.sync.dma_start(mask_sb, mask_d)
            nc.sync.dma_start(mask8_sb, mask8_d)

            # ---------------- QK projection (fp8 DoubleRow) -----------------
            qk_cache = {}

            def qk_proj_chunks(h):
                """Generator emitting 16 single-PSUM chunks (8 DR matmuls +
                one ACT identity each) of head h's Q/K projection.  Chunks
                alternate the two mm PSUM buffers, so chunk c+1 never waits
                on chunk c's ACT drain."""
                wq, wk = head_w[h]
                qT = qk_pool.tile([128, nqb, 512], bf16, tag="qT", name=f"qT_{h}")
                kT = qk_pool.tile([128, nqb, 512], bf16, tag="kT", name=f"kT_{h}")
                qk_cache[h] = (qT, kT)
                for w_t, dst, b_sb in ((wq, qT, qb_sb), (wk, kT, kb_sb)):
                    for pb in range(nqb):
                        ps = mm_ps.tile(
                            [128, 512], f32, tag="mm",
                            name=f"{dst.name[:2]}_{h}_{pb}",
                        )
                        for m2 in range(nt // 2):
                            lhs = w_t[:, m2, :] if USE_SWI else w_t[:, 2 * m2 : 2 * m2 + 2, :]
                            nc.tensor.matmul(
                                ps,
                                lhs,
                                xt8[:, 2 * m2 : 2 * m2 + 2,
                                    pb * 512 : (pb + 1) * 512],
                                start=(m2 == 0),
                                stop=(m2 == nt // 2 - 1),
                                perf_mode=DR,
                            )
                        nc.scalar.activation(
                            dst[:, pb, :],
                            ps,
                            AF.Identity,
                            bias=b_sb[:, h : h + 1],
                        )
                        yield
                head_w.pop(h, None)

            def run_all(gen):
                if gen is not None:
                    for _ in gen:
                        pass

            run_all(qk_proj_chunks(0))

            # ---------------- V projection (bf16), fp8 + bf16 stores --------
            p1_pools = [mm_ps, pv_ps]
            for p_i in range(npt):
                for blk in range(nblk):
                    pidx = (p_i * nblk + blk) % 2
                    vps = p1_pools[pidx].tile(
                        [128, 512], f32, tag=["mm", "pv"][pidx],
                        name=f"vps_{p_i}_{blk}",
                    )
                    for m in range(nt):
                        nc.tensor.matmul(
                            vps,
                            xt[:, m, p_i * 128 : (p_i + 1) * 128],
                            vw[:, m, blk * 512 : (blk + 1) * 512],
                            start=(m == 0),
                            stop=(m == nt - 1),
                        )
                    cols = slice(blk * 512, (blk + 1) * 512)
                    nc.vector.scalar_tensor_tensor(
                        vall8[:, p_i, cols], vps, SV, vb32_sb[:, cols], MULT, ADD
                    )
                    if p_i < kt_per_qb:
                        nc.vector.tensor_add(
                            vallb[:, p_i, cols], vps, vb_sb[:, cols]
                        )
            vw_pool.release()
            xt_pool.release()

            att_pool = tc.alloc_tile_pool(name="attp", bufs=1, side="right")
            pt_pool = tc.alloc_tile_pool(name="ptp", bufs=3)
            ptb_pool = tc.alloc_tile_pool(name="ptbp", bufs=2)
            acc_pool = tc.alloc_tile_pool(name="accp", bufs=3)
            rc_pool = tc.alloc_tile_pool(name="rcp", bufs=2)
            rb_pool = tc.alloc_tile_pool(name="rbp", bufs=2)
            attall = att_pool.tile([128, n_heads, seq], bf16)

            # ---------------- per-head attention ----------------------------
            # Head h's score blocks interleave head (h+1)'s QK-projection
            # chunks: when the score chain stalls on ACT/DVE, the in-order
            # PE queue still has projection matmuls to chew on.
            for h in range(n_heads):
                qT, kT = qk_cache.pop(h)
                nxt = h + 1
                if nxt < n_heads:
                    if nxt + 1 < n_heads:
                        fetch_w(nxt + 1)  # DMA for the head after next
                    ileave = qk_proj_chunks(nxt)
                else:
                    ileave = None

                for j in range(nqb):
                    nk = (j + 1) * kt_per_qb
                    aps = pv_ps.tile([128, 512], f32, tag="pv", name=f"aps_{h}_{j}")
                    acc = acc_pool.tile([128, 512], bf16, tag="acc", name=f"acc_{h}_{j}")
                    use8 = j >= 1
                    for ip in range(nk // 2):
                        i0, i1 = 2 * ip, 2 * ip + 1
                        s0 = 128 * (i0 - kt_per_qb * j) if i0 >= kt_per_qb * j else 0
                        s1 = 128 * (i1 - kt_per_qb * j) if i1 >= kt_per_qb * j else 0
                        stp = st_ps.tile(
                            [128, 2, 512], f32, tag="st", name=f"stp_{h}_{j}_{ip}"
                        )
                        for t, i in ((0, i0), (1, i1)):
                            nc.tensor.matmul(
                                stp[:, t, s0:512],
                                kT[:, i // 4, (i % 4) * 128 : (i % 4 + 1) * 128],
                                qT[:, j, s0:512],
                                start=True,
                                stop=True,
                            )
                        if use8:
                            ptile = pt_pool.tile(
                                [128, 2, 512], fp8, tag="pt", name=f"pt_{h}_{j}_{ip}"
                            )
                            msk = mask8_sb
                        else:
                            ptile = ptb_pool.tile(
                                [128, 2, 512], bf16, tag="ptb", name=f"pt_{h}_{j}_{ip}"
                            )
                            msk = mask_sb
                        nc.scalar.activation(
                            ptile[:, :, s0:512], stp[:, :, s0:512], AF.Exp,
                            scale=exp_scale,
                        )
                        for t, s in ((0, s0), (1, s1)):
                            if 2 * ip + t >= kt_per_qb * j:
                                nc.vector.tensor_mul(
                                    ptile[:, t, s0:512],
                                    ptile[:, t, s0:512],
                                    msk[:, 384 - (s - s0) : 896 - s],
                                )
                        if ip == 0:
                            nc.vector.tensor_add(
                                acc, ptile[:, 0, :], ptile[:, 1, :]
                            )
                        else:
                            nc.vector.tensor_add(
                                acc[:, s0:512], acc[:, s0:512], ptile[:, 0, s0:512]
                            )
                            nc.vector.tensor_add(
                                acc[:, s0:512], acc[:, s0:512], ptile[:, 1, s0:512]
                            )
                        if use8:
                            nc.tensor.matmul(
                                aps[:, s0:512],
                                vall8[:, i0 : i0 + 2, h * 128 : (h + 1) * 128],
                                ptile[:, :, s0:512],
                                start=(ip == 0),
                                stop=(ip == nk // 2 - 1),
                                perf_mode=DRPV,
                            )
                        else:
                            for t, (i, s) in ((0, (i0, s0)), (1, (i1, s1))):
                                nc.tensor.matmul(
                                    aps[:, s:512],
                                    vallb[:, i, h * 128 : (h + 1) * 128],
                                    ptile[:, t, s:512],
                                    start=(i == 0),
                                    stop=(i == nk - 1),
                                )
                    dnr = rb_pool.tile([128, 512], f32, tag="rb", name=f"dnr_{h}_{j}")
                    nc.gpsimd.partition_all_reduce(
                        dnr, acc, 128, bass_isa.ReduceOp.add
                    )
                    rc = rc_pool.tile([128, 512], f32, tag="rc", name=f"rc_{h}_{j}")
                    nc.vector.reciprocal_approx_fast(rc, dnr)
                    dst = attall[:, h, j * 512 : (j + 1) * 512]
                    if use8:
                        nc.vector.scalar_tensor_tensor(
                            dst, aps, 1.0 / SV, rc, MULT, MULT
                        )
                    else:
                        nc.vector.tensor_mul(dst, aps, rc)
                    if ileave is not None:
                        # 4 projection chunks of head h+1 after each j block
                        for _ in range(4):
                            next(ileave, None)
            rb_pool.release()
            rc_pool.release()
            acc_pool.release()
            ptb_pool.release()
            pt_pool.release()
            qk_pool.release()
            w_pool.release()
            vb16_pool.release()
            v8_pool.release()
            xt8_pool.release()

            # ---------------- output projection -----------------------------
            ow_pool = tc.alloc_tile_pool(name="owp", bufs=2 * n_heads, side="right")
            osb_pool = tc.alloc_tile_pool(name="osbp", bufs=4, side="right")
            for mb in range(d_model // 512):
                ows = []
                for h in range(n_heads):
                    owt = ow_pool.tile([128, 512], bf16, tag="ow", name=f"ow_{mb}_{h}")
                    nc.sync.dma_start(owt, ow_d[h][:, mb * 512 : (mb + 1) * 512])
                    ows.append(owt)
                for p_i in range(npt):
                    ops = mm_ps.tile([128, 512], f32, tag="mm", name=f"ops_{mb}_{p_i}")
                    for h in range(n_heads):
                        nc.tensor.matmul(
                            ops,
                            attall[:, h, p_i * 128 : (p_i + 1) * 128],
                            ows[h],
                            start=(h == 0),
                            stop=(h == n_heads - 1),
                        )
                    osb = osb_pool.tile([128, 512], f32, tag="osb", name=f"osb_{mb}_{p_i}")
                    nc.scalar.copy(osb, ops)
                    nc.sync.dma_start(
                        out_d[p_i * 128 : (p_i + 1) * 128, mb * 512 : (mb + 1) * 512],
                        osb,
                    )
            osb_pool.release()
            ow_pool.release()
            att_pool.release()
            consts.release()
            pv_ps.release()
            st_ps.release()
            mm_ps.release()
    nc.finalize()
    return nc


def _interleave_pairs(w):
    """[nt, 128, 128] -> [nt//2, 128, 256] SwInterleave layout:
    flat[:, 2j] = A[:, 127-j], flat[:, 2j+1] = B[:, 127-j]."""
    nt = w.shape[0]
    out = np.empty((nt // 2, 128, 256), w.dtype)
    for t2 in range(nt // 2):
        a, b = w[2 * t2], w[2 * t2 + 1]
        out[t2, :, 0::2] = a[:, ::-1]
        out[t2, :, 1::2] = b[:, ::-1]
    return out


def make_core_inputs(x_b, Qw, Qb, Kw, Kb, Vw, Vb, Ow, seq, d_model, n_heads):
    """Host-side prep of one core's input map.

    x_b: [seq, d_model] fp32.  Qw/Kw/Vw: [n_heads, d_model, 128].
    Qb/Kb/Vb: [n_heads, 128].  Ow: [n_heads, 128, d_model].
    """
    nt = d_model // 128
    nhd = n_heads * DH

    # xT as [128(m_in), nt, seq]
    xTr = x_b.T.reshape(nt, 128, seq).transpose(1, 0, 2)
    xt = np.ascontiguousarray(xTr.astype(BF16))
    xt8 = np.ascontiguousarray((xTr * SX).astype(FP8))
    qw_t = (Qw * SW).reshape(n_heads, nt, 128, 128)
    kw_t = (Kw * SW).reshape(n_heads, nt, 128, 128)
    if USE_SWI:
        qw = np.ascontiguousarray(
            np.stack([_interleave_pairs(qw_t[h].astype(FP8)) for h in range(n_heads)])
        ).transpose(0, 2, 1, 3)
        qw = np.ascontiguousarray(qw)
        kw = np.ascontiguousarray(
            np.stack([_interleave_pairs(kw_t[h].astype(FP8)) for h in range(n_heads)])
        ).transpose(0, 2, 1, 3)
        kw = np.ascontiguousarray(kw)
    else:
        qw = np.ascontiguousarray(qw_t.transpose(0, 2, 1, 3).astype(FP8))
        kw = np.ascontiguousarray(kw_t.transpose(0, 2, 1, 3).astype(FP8))
    # v weights as [128(m_in), nt, (h d)]
    vw = np.ascontiguousarray(
        Vw.transpose(1, 0, 2)
        .reshape(d_model, nhd)
        .reshape(nt, 128, nhd)
        .transpose(1, 0, 2)
        .astype(BF16)
    )
    ow = np.ascontiguousarray(Ow.astype(BF16))
    qb = np.ascontiguousarray((Qb * SX * SW).T.astype(np.float32))
    kb = np.ascontiguousarray((Kb * SX * SW).T.astype(np.float32))
    vb = np.ascontiguousarray(
        np.broadcast_to(Vb.reshape(1, nhd), (128, nhd)).astype(np.float32)
    )
    vb32 = np.ascontiguousarray(vb * SV)
    r = np.arange(128, dtype=np.int64)[:, None]
    u = np.arange(896, dtype=np.int64)[None, :]
    mask = (r <= u - 384).astype(BF16)
    mask8 = mask.astype(FP8)
    return {
        "xt": xt,
        "xt8": xt8,
        "qw": qw,
        "kw": kw,
        "vw": vw,
        "ow": ow,
        "qb": qb,
        "kb": kb,
        "vb": vb,
        "vb32": vb32,
        "mask": mask,
        "mask8": mask8,
    }


_NC_CACHE = None


def kernel(**inputs):
    global _NC_CACHE
    from concourse.bass_utils import run_bass_kernel_spmd

    x = np.asarray(inputs["x"], np.float32)
    Q_w = np.asarray(inputs["Q_w"], np.float32)
    Q_b = np.asarray(inputs["Q_b"], np.float32)
    K_w = np.asarray(inputs["K_w"], np.float32)
    K_b = np.asarray(inputs["K_b"], np.float32)
    V_w = np.asarray(inputs["V_w"], np.float32)
    V_b = np.asarray(inputs["V_b"], np.float32)
    O_w = np.asarray(inputs["O_w"], np.float32)
    O_b = np.asarray(inputs["O_b"], np.float32)

    B, seq, d_model = x.shape

    if _NC_CACHE is None:
        _NC_CACHE = build_program(seq=seq, d_model=d_model, n_heads=NH_LOC)
    nc = _NC_CACHE

    in_maps = []
    for c in range(N_CORES):
        b = c // 2
        g = c % 2
        hs = slice(g * NH_LOC, (g + 1) * NH_LOC)
        in_maps.append(
            make_core_inputs(
                x[b], Q_w[hs], Q_b[hs], K_w[hs], K_b[hs], V_w[hs], V_b[hs],
                O_w[hs], seq, d_model, NH_LOC,
            )
        )

    res = run_bass_kernel_spmd(nc, in_maps, core_ids=list(range(N_CORES)))
    out = np.empty((B, seq, d_model), np.float32)
    for b in range(B):
        out[b] = res.results[2 * b]["out"] + res.results[2 * b + 1]["out"] + O_b[None, :]
    return out


# revision 22
# speedup vs baseline: 1.2353x; 1.1491x over previous
"""Trainium2 Bass kernel for a 16-head causal attention block.

Problem: B=4, S=2048, D_MODEL=2048, N_HEADS=16, D_HEAD=128, fp32 I/O.

Sharding (8 cores): core c handles batch b = c//2 and head-group g = c%2
(8 heads each).  Each core computes its heads' attention and the partial
output projection (sum over its 8 heads) for its batch; the host sums the
two head-group partials per batch and adds the output bias.

Per-core dataflow (fp32 PSUM accumulation everywhere):
  emission order: h0-QK-proj, h1-QK-proj, V-proj(all), h0..h7 scores
  (h2+ include their QK proj), output projection.  This front-loads two
  heads' QK work so the PE starts on a 4MB xt8 DMA instead of the full
  16MB stream, and stretches the ACT(exp) window.

  QK proj (fp8 DoubleRow): qT/kT [dh, seq] bf16 via ACT identity+bias.
  scores, per q-block j (512 wide), k-tile PAIRS (2x128):
    ST pair [128,2,512] fp32 PSUM   (2 matmuls, bf16 operands)
    PT pair = exp(ST)               (ONE pair-wide ACT op)
      j==0 -> bf16 PT; j>=1 -> fp8e4 PT
    causal mask per diagonal sub-tile (DVE mul)
    acc += PT sub-tiles             (DVE, bf16 accumulator)
    j==0: attnT += v_tile^T (x) PT        (per-tile bf16 matmul)
    j>=1: attnT += v8_pair^T (x) PT_pair  (fp8 DoubleRow matmul)
  denom = partition_all_reduce(acc)  (GPSIMD, bf16 in / fp32 out)
  recip = approx 1/denom             (DVE)
  attall = aps * recip (bf16; j>=1 folds the 1/32 v-scale via
           scalar_tensor_tensor)
  V proj keeps x in bf16 (fp8 V projection fails the 2e-2 gate); the
  STORED v is fp8(32*v) for j>=1 plus bf16 tiles 0..3 for j==0 --
  hybrid validated at rel_err 3.4e-3 == baseline.
  out[p, m] = sum_h attall_h^T (x) Ow_h  (bf16 matmuls) -> fp32 -> DRAM
"""

import math
import sys

import numpy as np
import ml_dtypes

for _p in ("/opt/trn_rl_repo", "/root/.axon_site/_ro/trn_rl_repo"):
    if _p not in sys.path:
        sys.path.insert(0, _p)

BF16 = ml_dtypes.bfloat16
FP8 = ml_dtypes.float8_e4m3

# fp8(e4m3) DoubleRow for the Q/K projections; logits are tiny so fp8
# noise there is harmless.  SX/SW are undone inside the exp scale.
SX = 8.0
SW = 2000.0
SV = 32.0   # stored-v scale for the fp8 PV path
SA = 64.0   # stored-attn scale for the fp8 O-proj path
SO = 200.0  # Ow scale for the fp8 O-proj path

S_FULL = 2048
D_FULL = 2048
NH_LOC = 8  # heads per core
DH = 128
QB = 512  # q block width
N_CORES = 8

# DoubleRowSwInterleave for QK proj: weights pre-interleaved host-side so
# LDWEIGHTS reads contiguously (FWL-speed) instead of the +72% DR pattern.
USE_SWI = False


def build_program(seq=S_FULL, d_model=D_FULL, n_heads=NH_LOC, loop_n=1):
    import concourse.tile as tile
    from concourse import bacc, bass_isa, mybir

    # Calibrate the scheduler's PE cost model to measured HW: matmuls
    # stream at ~0.5 ns/col (P0 downclock) and fp8-DoubleRow runs at the
    # same per-column rate as bf16 (the model prices DR at 0.5 cyc/row).
    # PE_CYCLE=1/1.2GHz makes bf16 ~1.5x over-priced and DR ~1.3x
    # under-priced, which ranks PE/DVE/ACT correctly for the head loop --
    # the Tile list-scheduler then orders the streams for a PE-bound
    # machine instead of a DVE-bound one.
    from concourse import hw_specs

    hw_specs.TRN2Spec.PE_CYCLE = 1e9 / 1.2e9

    f32 = mybir.dt.float32
    bf16 = mybir.dt.bfloat16
    fp8 = mybir.dt.float8e4
    AF = mybir.ActivationFunctionType
    DR = (
        mybir.MatmulPerfMode.DoubleRowSwInterleave
        if USE_SWI
        else mybir.MatmulPerfMode.DoubleRow
    )
    DRPV = mybir.MatmulPerfMode.DoubleRow
    MULT = mybir.AluOpType.mult
    ADD = mybir.AluOpType.add

    nt = d_model // 128  # contraction (d_model) tiles
    npt = seq // 128  # seq tiles (p / k)
    nqb = seq // QB  # q blocks
    kt_per_qb = QB // 128  # 4
    nhd = n_heads * DH  # concatenated head width
    nblk = nhd // 512  # 512-wide chunks of (h, d)

    nc = bacc.Bacc(
        "TRN2", target_bir_lowering=False, debug=False, enable_asserts=False
    )

    exp_scale = 1.0 / ((SX * SW) ** 2 * math.sqrt(DH))
    xt_d = nc.dram_tensor("xt", [128, nt, seq], bf16, kind="ExternalInput").ap()
    xt8_d = nc.dram_tensor("xt8", [128, nt, seq], fp8, kind="ExternalInput").ap()
    if USE_SWI:
        qw_d = nc.dram_tensor(
            "qw", [n_heads, 128, nt // 2, 256], fp8, kind="ExternalInput"
        ).ap()
        kw_d = nc.dram_tensor(
            "kw", [n_heads, 128, nt // 2, 256], fp8, kind="ExternalInput"
        ).ap()
    else:
        qw_d = nc.dram_tensor(
            "qw", [n_heads, 128, nt, 128], fp8, kind="ExternalInput"
        ).ap()
        kw_d = nc.dram_tensor(
            "kw", [n_heads, 128, nt, 128], fp8, kind="ExternalInput"
        ).ap()
    vw_d = nc.dram_tensor("vw", [128, nt, nhd], bf16, kind="ExternalInput").ap()
    vw8_d = nc.dram_tensor("vw8", [128, nt, nhd], fp8, kind="ExternalInput").ap()
    ow_d = nc.dram_tensor("ow", [n_heads, 128, d_model], bf16, kind="ExternalInput").ap()
    ow8_d = nc.dram_tensor("ow8", [128, n_heads, d_model], fp8, kind="ExternalInput").ap()
    qb_d = nc.dram_tensor("qb", [128, n_heads], f32, kind="ExternalInput").ap()
    kb_d = nc.dram_tensor("kb", [128, n_heads], f32, kind="ExternalInput").ap()
    vb_d = nc.dram_tensor("vb", [128, nhd], f32, kind="ExternalInput").ap()
    vb32_d = nc.dram_tensor("vb32", [128, nhd], f32, kind="ExternalInput").ap()
    mask_d = nc.dram_tensor("mask", [128, 896], bf16, kind="ExternalInput").ap()
    mask8_d = nc.dram_tensor("mask8", [128, 896], fp8, kind="ExternalInput").ap()
    out_d = nc.dram_tensor("out", [seq, d_model], f32, kind="ExternalOutput").ap()

    from concourse import library_config

    with tile.TileContext(nc) as tc:
        nc.gpsimd.load_library(library_config.attn)
        for _rep in range(loop_n):
            # PSUM pools: 2 + 4 + 2 = 8 banks
            mm_ps = tc.alloc_tile_pool(name="mmps", bufs=2, space="PSUM")
            st_ps = tc.alloc_tile_pool(name="stps", bufs=2, space="PSUM")
            pv_ps = tc.alloc_tile_pool(name="pvps", bufs=2, space="PSUM")

            consts = tc.alloc_tile_pool(name="consts", bufs=1)
            xt8_pool = tc.alloc_tile_pool(name="xt8p", bufs=1)
            v8_pool = tc.alloc_tile_pool(name="v8p", bufs=1)
            vb16_pool = tc.alloc_tile_pool(name="vb16p", bufs=1)
            w_pool = tc.alloc_tile_pool(name="wp", bufs=3)
            qk_pool = tc.alloc_tile_pool(name="qkp", bufs=3)
            xt_pool = tc.alloc_tile_pool(name="xtp", bufs=1)
            vw_pool = tc.alloc_tile_pool(name="vwp", bufs=1)

            mask_sb = consts.tile([128, 896], bf16)
            mask8_sb = consts.tile([128, 896], fp8)
            qb_sb = consts.tile([128, n_heads], f32)
            kb_sb = consts.tile([128, n_heads], f32)
            vb_sb = consts.tile([128, nhd], f32)
            vb32_sb = consts.tile([128, nhd], f32)

            xt8 = xt8_pool.tile([128, nt, seq], fp8)
            vall8 = v8_pool.tile([128, npt, nhd], fp8)
            vallb = vb16_pool.tile([128, kt_per_qb, nhd], bf16)
            xt = xt_pool.tile([128, nt, seq], bf16)
            vw = vw_pool.tile([128, nt, nhd], bf16)

            # ---------------- DMA: xt8 + early-head weights first ----------
            for t in range(nt):
                nc.sync.dma_start(xt8[:, t, :], xt8_d[:, t, :])
            nc.sync.dma_start(qb_sb, qb_d)
            nc.sync.dma_start(kb_sb, kb_d)
            head_w = {}

            def fetch_w(h):
                wq = w_pool.tile(list(qw_d[h].shape), fp8, tag="wq", name=f"wq_{h}")
                nc.sync.dma_start(wq, qw_d[h])
                wk = w_pool.tile(list(kw_d[h].shape), fp8, tag="wk", name=f"wk_{h}")
                nc.sync.dma_start(wk, kw_d[h])
                head_w[h] = (wq, wk)

            for h in (0, 1):
                fetch_w(h)
            for t in range(nt):
                nc.sync.dma_start(xt[:, t, :], xt_d[:, t, :])
                nc.sync.dma_start(vw[:, t, :], vw_d[:, t, :])
            nc.sync.dma_start(vb_sb, vb_d)
            nc.sync.dma_start(vb32_sb, vb32_d)
            nc.sync.dma_start(mask_sb, mask_d)
            nc.sync.dma_start(mask8_sb, mask8_d)

            # ---------------- QK projection (fp8 DoubleRow) -----------------
            qk_cache = {}

            def qk_proj_chunks(h):
                """Generator emitting 16 single-PSUM chunks (8 DR matmuls +
                one ACT identity each) of head h's Q/K projection.  Chunks
                alternate the two mm PSUM buffers, so chunk c+1 never waits
                on chunk c's ACT drain."""
                wq, wk = head_w[h]
                qT = qk_pool.tile([128, nqb, 512], bf16, tag="qT", name=f"qT_{h}")
                kT = qk_pool.tile([128, nqb, 512], bf16, tag="kT", name=f"kT_{h}")
                qk_cache[h] = (qT, kT)
                for w_t, dst, b_sb in ((wq, qT, qb_sb), (wk, kT, kb_sb)):
                    for pb in range(nqb):
                        ps = mm_ps.tile(
                            [128, 512], f32, tag="mm",
                            name=f"{dst.name[:2]}_{h}_{pb}",
                        )
                        for m2 in range(nt // 2):
                            lhs = w_t[:, m2, :] if USE_SWI else w_t[:, 2 * m2 : 2 * m2 + 2, :]
                            nc.tensor.matmul(
                                ps,
                                lhs,
                                xt8[:, 2 * m2 : 2 * m2 + 2,
                                    pb * 512 : (pb + 1) * 512],
                                start=(m2 == 0),
                                stop=(m2 == nt // 2 - 1),
                                perf_mode=DR,
                            )
                        nc.scalar.activation(
                            dst[:, pb, :],
                            ps,
                            AF.Identity,
                            bias=b_sb[:, h : h + 1],
                        )
                        yield
                head_w.pop(h, None)

            def run_all(gen):
                if gen is not None:
                    for _ in gen:
                        pass

            run_all(qk_proj_chunks(0))

            # ---------------- V projection, row-hybrid precision ------------
            # k-tiles 0..3 (feeding q-block 0) in bf16; k-tiles 4..15 via
            # fp8 DoubleRow -- near-uniform attention over >=512 keys
            # averages the fp8 noise away (validated rel_err 8.2e-3).
            p1_pools = [mm_ps, pv_ps]
            for p_i in range(kt_per_qb):
                for blk in range(nblk):
                    pidx = (p_i * nblk + blk) % 2
                    vps = p1_pools[pidx].tile(
                        [128, 512], f32, tag=["mm", "pv"][pidx],
                        name=f"vps_{p_i}_{blk}",
                    )
                    for m in range(nt):
                        nc.tensor.matmul(
                            vps,
                            xt[:, m, p_i * 128 : (p_i + 1) * 128],
                            vw[:, m, blk * 512 : (blk + 1) * 512],
                            start=(m == 0),
                            stop=(m == nt - 1),
                        )
                    cols = slice(blk * 512, (blk + 1) * 512)
                    nc.vector.scalar_tensor_tensor(
                        vall8[:, p_i, cols], vps, SV, vb32_sb[:, cols], MULT, ADD
                    )
                    nc.vector.tensor_add(
                        vallb[:, p_i, cols], vps, vb_sb[:, cols]
                    )
            vw_pool.release()
            xt_pool.release()

            vw8_pool = tc.alloc_tile_pool(name="vw8p", bufs=1)
            vw8 = vw8_pool.tile([128, nt, nhd], fp8)
            for t in range(nt):
                nc.sync.dma_start(vw8[:, t, :], vw8_d[:, t, :])
            for p_i in range(kt_per_qb, npt):
                for blk in range(nblk):
                    pidx = (p_i * nblk + blk) % 2
                    vps = p1_pools[pidx].tile(
                        [128, 512], f32, tag=["mm", "pv"][pidx],
                        name=f"vps_{p_i}_{blk}",
                    )
                    for m2 in range(nt // 2):
                        nc.tensor.matmul(
                            vps,
                            xt8[:, 2 * m2 : 2 * m2 + 2,
                                p_i * 128 : (p_i + 1) * 128],
                            vw8[:, 2 * m2 : 2 * m2 + 2,
                                blk * 512 : (blk + 1) * 512],
                            start=(m2 == 0),
                            stop=(m2 == nt // 2 - 1),
                            perf_mode=DRPV,
                        )
                    cols = slice(blk * 512, (blk + 1) * 512)
                    nc.vector.scalar_tensor_tensor(
                        vall8[:, p_i, cols], vps, SV / (SX * SW),
                        vb32_sb[:, cols], MULT, ADD,
                    )
            vw8_pool.release()

            att_pool = tc.alloc_tile_pool(name="attp", bufs=1, side="right")
            pt_pool = tc.alloc_tile_pool(name="ptp", bufs=3)
            ptb_pool = tc.alloc_tile_pool(name="ptbp", bufs=2)
            acc_pool = tc.alloc_tile_pool(name="accp", bufs=3)
            rc_pool = tc.alloc_tile_pool(name="rcp", bufs=2)
            rb_pool = tc.alloc_tile_pool(name="rbp", bufs=2)
            # row-hybrid attn store: rows 0..511 (j0) bf16, rows 512+ fp8*SA
            attallb = att_pool.tile([128, n_heads, QB], bf16)
            attall8 = att_pool.tile([128, n_heads, seq], fp8)

            # ---------------- per-head attention ----------------------------
            # Head h's score blocks interleave head (h+1)'s QK-projection
            # chunks: when the score chain stalls on ACT/DVE, the in-order
            # PE queue still has projection matmuls to chew on.
            for h in range(n_heads):
                qT, kT = qk_cache.pop(h)
                nxt = h + 1
                if nxt < n_heads:
                    if nxt + 1 < n_heads:
                        fetch_w(nxt + 1)  # DMA for the head after next
                    ileave = qk_proj_chunks(nxt)
                else:
                    ileave = None

                for j in range(nqb):
                    nk = (j + 1) * kt_per_qb
                    aps = pv_ps.tile([128, 512], f32, tag="pv", name=f"aps_{h}_{j}")
                    acc = acc_pool.tile([128, 512], bf16, tag="acc", name=f"acc_{h}_{j}")
                    use8 = j >= 1
                    for ip in range(nk // 2):
                        i0, i1 = 2 * ip, 2 * ip + 1
                        s0 = 128 * (i0 - kt_per_qb * j) if i0 >= kt_per_qb * j else 0
                        s1 = 128 * (i1 - kt_per_qb * j) if i1 >= kt_per_qb * j else 0
                        stp = st_ps.tile(
                            [128, 2, 512], f32, tag="st", name=f"stp_{h}_{j}_{ip}"
                        )
                        for t, i in ((0, i0), (1, i1)):
                            nc.tensor.matmul(
                                stp[:, t, s0:512],
                                kT[:, i // 4, (i % 4) * 128 : (i % 4 + 1) * 128],
                                qT[:, j, s0:512],
                                start=True,
                                stop=True,
                            )
                        if use8:
                            ptile = pt_pool.tile(
                                [128, 2, 512], fp8, tag="pt", name=f"pt_{h}_{j}_{ip}"
                            )
                            msk = mask8_sb
                        else:
                            ptile = ptb_pool.tile(
                                [128, 2, 512], bf16, tag="ptb", name=f"pt_{h}_{j}_{ip}"
                            )
                            msk = mask_sb
                        nc.scalar.activation(
                            ptile[:, :, s0:512], stp[:, :, s0:512], AF.Exp,
                            scale=exp_scale,
                        )
                        for t, s in ((0, s0), (1, s1)):
                            if 2 * ip + t >= kt_per_qb * j:
                                nc.vector.tensor_mul(
                                    ptile[:, t, s0:512],
                                    ptile[:, t, s0:512],
                                    msk[:, 384 - (s - s0) : 896 - s],
                                )
                        if ip == 0:
                            nc.vector.tensor_add(
                                acc, ptile[:, 0, :], ptile[:, 1, :]
                            )
                        else:
                            nc.vector.tensor_add(
                                acc[:, s0:512], acc[:, s0:512], ptile[:, 0, s0:512]
                            )
                            nc.vector.tensor_add(
                                acc[:, s0:512], acc[:, s0:512], ptile[:, 1, s0:512]
                            )
                        if use8:
                            nc.tensor.matmul(
                                aps[:, s0:512],
                                vall8[:, i0 : i0 + 2, h * 128 : (h + 1) * 128],
                                ptile[:, :, s0:512],
                                start=(ip == 0),
                                stop=(ip == nk // 2 - 1),
                                perf_mode=DRPV,
                            )
                        else:
                            for t, (i, s) in ((0, (i0, s0)), (1, (i1, s1))):
                                nc.tensor.matmul(
                                    aps[:, s:512],
                                    vallb[:, i, h * 128 : (h + 1) * 128],
                                    ptile[:, t, s:512],
                                    start=(i == 0),
                                    stop=(i == nk - 1),
                                )
                    dnr = rb_pool.tile([128, 512], f32, tag="rb", name=f"dnr_{h}_{j}")
                    nc.gpsimd.partition_all_reduce(
                        dnr, acc, 128, bass_isa.ReduceOp.add
                    )
                    rc = rc_pool.tile([128, 512], f32, tag="rc", name=f"rc_{h}_{j}")
                    nc.vector.reciprocal_approx_fast(rc, dnr)
                    if use8:
                        nc.vector.scalar_tensor_tensor(
                            attall8[:, h, j * 512 : (j + 1) * 512],
                            aps, SA / SV, rc, MULT, MULT,
                        )
                    else:
                        nc.vector.tensor_mul(attallb[:, h, :], aps, rc)
                    if ileave is not None:
                        # 4 projection chunks of head h+1 after each j block
                        for _ in range(4):
                            next(ileave, None)
            rb_pool.release()
            rc_pool.release()
            acc_pool.release()
            ptb_pool.release()
            pt_pool.release()
            qk_pool.release()
            w_pool.release()
            vb16_pool.release()
            v8_pool.release()
            xt8_pool.release()

            # ------------- output projection (row-hybrid) -------------------
            # rows 0..511 (p_i 0..3): bf16 attall x bf16 Ow;
            # rows 512+ (p_i 4..15): fp8 DoubleRow over head pairs.
            ow_pool = tc.alloc_tile_pool(name="owp", bufs=2 * n_heads, side="right")
            ow8_pool = tc.alloc_tile_pool(name="ow8p", bufs=2, side="right")
            osb_pool = tc.alloc_tile_pool(name="osbp", bufs=4, side="right")
            for mb in range(d_model // 512):
                ows = []
                for h in range(n_heads):
                    owt = ow_pool.tile([128, 512], bf16, tag="ow", name=f"ow_{mb}_{h}")
                    nc.sync.dma_start(owt, ow_d[h][:, mb * 512 : (mb + 1) * 512])
                    ows.append(owt)
                ow8t = ow8_pool.tile(
                    [128, n_heads, 512], fp8, tag="ow8", name=f"ow8_{mb}"
                )
                nc.sync.dma_start(ow8t, ow8_d[:, :, mb * 512 : (mb + 1) * 512])
                for p_i in range(npt):
                    ops = mm_ps.tile([128, 512], f32, tag="mm", name=f"ops_{mb}_{p_i}")
                    if p_i < kt_per_qb:
                        for h in range(n_heads):
                            nc.tensor.matmul(
                                ops,
                                attallb[:, h,
                                        p_i * 128 : (p_i + 1) * 128],
                                ows[h],
                                start=(h == 0),
                                stop=(h == n_heads - 1),
                            )
                    else:
                        for h2 in range(n_heads // 2):
                            nc.tensor.matmul(
                                ops,
                                attall8[:, 2 * h2 : 2 * h2 + 2,
                                        p_i * 128 : (p_i + 1) * 128],
                                ow8t[:, 2 * h2 : 2 * h2 + 2, :],
                                start=(h2 == 0),
                                stop=(h2 == n_heads // 2 - 1),
                                perf_mode=DRPV,
                            )
                    osb = osb_pool.tile([128, 512], f32, tag="osb", name=f"osb_{mb}_{p_i}")
                    if p_i < kt_per_qb:
                        nc.scalar.copy(osb, ops)
                    else:
                        nc.scalar.activation(
                            osb, ops, AF.Identity, scale=1.0 / (SA * SO)
                        )
                    nc.sync.dma_start(
                        out_d[p_i * 128 : (p_i + 1) * 128, mb * 512 : (mb + 1) * 512],
                        osb,
                    )
            osb_pool.release()
            ow8_pool.release()
            ow_pool.release()
            att_pool.release()
            consts.release()
            pv_ps.release()
            st_ps.release()
            mm_ps.release()
    nc.finalize()
    return nc


def _interleave_pairs(w):
    """[nt, 128, 128] -> [nt//2, 128, 256] SwInterleave layout:
    flat[:, 2j] = A[:, 127-j], flat[:, 2j+1] = B[:, 127-j]."""
    nt = w.shape[0]
    out = np.empty((nt // 2, 128, 256), w.dtype)
    for t2 in range(nt // 2):
        a, b = w[2 * t2], w[2 * t2 + 1]
        out[t2, :, 0::2] = a[:, ::-1]
        out[t2, :, 1::2] = b[:, ::-1]
    return out


def make_core_inputs(x_b, Qw, Qb, Kw, Kb, Vw, Vb, Ow, seq, d_model, n_heads):
    """Host-side prep of one core's input map.

    x_b: [seq, d_model] fp32.  Qw/Kw/Vw: [n_heads, d_model, 128].
    Qb/Kb/Vb: [n_heads, 128].  Ow: [n_heads, 128, d_model].
    """
    nt = d_model // 128
    nhd = n_heads * DH

    # xT as [128(m_in), nt, seq]
    xTr = x_b.T.reshape(nt, 128, seq).transpose(1, 0, 2)
    xt = np.ascontiguousarray(xTr.astype(BF16))
    xt8 = np.ascontiguousarray((xTr * SX).astype(FP8))
    qw_t = (Qw * SW).reshape(n_heads, nt, 128, 128)
    kw_t = (Kw * SW).reshape(n_heads, nt, 128, 128)
    if USE_SWI:
        qw = np.ascontiguousarray(
            np.stack([_interleave_pairs(qw_t[h].astype(FP8)) for h in range(n_heads)])
        ).transpose(0, 2, 1, 3)
        qw = np.ascontiguousarray(qw)
        kw = np.ascontiguousarray(
            np.stack([_interleave_pairs(kw_t[h].astype(FP8)) for h in range(n_heads)])
        ).transpose(0, 2, 1, 3)
        kw = np.ascontiguousarray(kw)
    else:
        qw = np.ascontiguousarray(qw_t.transpose(0, 2, 1, 3).astype(FP8))
        kw = np.ascontiguousarray(kw_t.transpose(0, 2, 1, 3).astype(FP8))
    # v weights as [128(m_in), nt, (h d)]
    vw = np.ascontiguousarray(
        Vw.transpose(1, 0, 2)
        .reshape(d_model, nhd)
        .reshape(nt, 128, nhd)
        .transpose(1, 0, 2)
        .astype(BF16)
    )
    vw8 = np.ascontiguousarray(
        (Vw * SW).transpose(1, 0, 2)
        .reshape(d_model, nhd)
        .reshape(nt, 128, nhd)
        .transpose(1, 0, 2)
        .astype(FP8)
    )
    ow = np.ascontiguousarray(Ow.astype(BF16))
    ow8 = np.ascontiguousarray((Ow * SO).transpose(1, 0, 2).astype(FP8))
    qb = np.ascontiguousarray((Qb * SX * SW).T.astype(np.float32))
    kb = np.ascontiguousarray((Kb * SX * SW).T.astype(np.float32))
    vb = np.ascontiguousarray(
        np.broadcast_to(Vb.reshape(1, nhd), (128, nhd)).astype(np.float32)
    )
    vb32 = np.ascontiguousarray(vb * SV)
    r = np.arange(128, dtype=np.int64)[:, None]
    u = np.arange(896, dtype=np.int64)[None, :]
    mask = (r <= u - 384).astype(BF16)
    mask8 = mask.astype(FP8)
    return {
        "xt": xt,
        "xt8": xt8,
        "qw": qw,
        "kw": kw,
        "vw": vw,
        "vw8": vw8,
        "ow": ow,
        "ow8": ow8,
        "qb": qb,
        "kb": kb,
        "vb": vb,
        "vb32": vb32,
        "mask": mask,
        "mask8": mask8,
    }


_NC_CACHE = None


def kernel(**inputs):
    global _NC_CACHE
    from concourse.bass_utils import run_bass_kernel_spmd

    x = np.asarray(inputs["x"], np.float32)
    Q_w = np.asarray(inputs["Q_w"], np.float32)
    Q_b = np.asarray(inputs["Q_b"], np.float32)
    K_w = np.asarray(inputs["K_w"], np.float32)
    K_b = np.asarray(inputs["K_b"], np.float32)
    V_w = np.asarray(inputs["V_w"], np.float32)
    V_b = np.asarray(inputs["V_b"], np.float32)
    O_w = np.asarray(inputs["O_w"], np.float32)
    O_b = np.asarray(inputs["O_b"], np.float32)

    B, seq, d_model = x.shape

    if _NC_CACHE is None:
        _NC_CACHE = build_program(seq=seq, d_model=d_model, n_heads=NH_LOC)
    nc = _NC_CACHE

    in_maps = []
    for c in range(N_CORES):
        b = c // 2
        g = c % 2
        hs = slice(g * NH_LOC, (g + 1) * NH_LOC)
        in_maps.append(
            make_core_inputs(
                x[b], Q_w[hs], Q_b[hs], K_w[hs], K_b[hs], V_w[hs], V_b[hs],
                O_w[hs], seq, d_model, NH_LOC,
            )
        )

    res = run_bass_kernel_spmd(nc, in_maps, core_ids=list(range(N_CORES)))
    out = np.empty((B, seq, d_model), np.float32)
    for b in range(B):
        out[b] = res.results[2 * b]["out"] + res.results[2 * b + 1]["out"] + O_b[None, :]
    return out
